# revision 1
# baseline (speedup 1.0000x reference)
"""Trainium2 Bass kernel for nn_SimpleMLP (segment-mean + 2-layer MLP).

reference:
  sums = segment_sum(x, batch, 4096); cnt = segment_sum(ones, batch, 4096)
  pooled = sums / max(cnt, 1);  out = gelu(pooled @ W1 + b1) @ W2 + b2

Distribution (8 cores, no collectives): `batch` is sorted, so core k owns
segments [512k, 512k+512). The host finds each core's row range by
searchsorted, hands core k a fixed-size row slab (zero-copy view) plus
bseg = batch - 512k as f32. On-device, each 128-row tile is turned into a
one-hot [rows x 128segs] matrix (iota + tensor_scalar is_equal) and
matmul'd (f32r) against the x tile with two appended ones-columns, so one
matmul accumulates both segment sums and counts into PSUM. Four static
128-segment windows per core; rows outside a window produce all-zero
one-hot rows, which makes slab padding/overlap masking free. Mean + MLP
(fp32 matmuls, hardware Gelu) run replicated per core on its 512 segments;
host concatenates the 8 [512, 256] outputs.
"""
import sys

sys.path.insert(0, "/opt/trn_rl_repo")

from contextlib import ExitStack

import numpy as np

import concourse.bacc as bacc
import concourse.mybir as mybir
import concourse.tile as tile
from concourse import bass_utils

F32 = mybir.dt.float32
F32R = mybir.dt.float32r
F16 = mybir.dt.float16

N = 1048576
H = 256
S = 4096
NCORES = 8
SEG_PC = S // NCORES          # 512 segments per core
G = 4                         # 128-seg windows per core
TPS = 16                      # 128-row tiles per supertile
SUP_ROWS = TPS * 128          # 2048
NSUP = 65
R_PAD = NSUP * SUP_ROWS       # 133120 rows per core slab

# window g covers bseg in [128g, 128g+128); processed supertile range
# [16g-1, 16g+18) covers data-dependent row drift (+/-417 rows measured)
# plus core-7's clamp shift (up to +2600 rows).
WLO = [max(16 * g - 1, 0) for g in range(G)]
WHI = [min(16 * g + 18, NSUP) for g in range(G)]

_nc_cache = None


def _build_nc(use_gelu=True):
    nc = bacc.Bacc("TRN2", target_bir_lowering=False, debug=False,
                   num_devices=NCORES)
    xs_d = nc.dram_tensor("xs", [R_PAD, H], F16, kind="ExternalInput")
    # bseg pre-transposed on host to the on-chip layout [p, st, b] so the
    # DMA is contiguous (the gather layout cost 77us in DMA descriptors)
    bs_d = nc.dram_tensor("bs", [128, NSUP * TPS], F16, kind="ExternalInput")
    # 1/max(cnt,1) per segment, host-computed: [p, g] for window g
    rcp_d = nc.dram_tensor("rcp", [128, G], F32, kind="ExternalInput")
    w1_d = nc.dram_tensor("w1", [H, H], F32, kind="ExternalInput")
    b1_d = nc.dram_tensor("b1", [H], F32, kind="ExternalInput")
    w2_d = nc.dram_tensor("w2", [H, H], F32, kind="ExternalInput")
    b2_d = nc.dram_tensor("b2", [H], F32, kind="ExternalInput")
    out_d = nc.dram_tensor("out", [SEG_PC, H], F32, kind="ExternalOutput")

    with tile.TileContext(nc) as tc, ExitStack() as ctx:
        const = ctx.enter_context(tc.tile_pool(name="const", bufs=1))
        xp = ctx.enter_context(tc.tile_pool(name="xp", bufs=6))
        ohp = ctx.enter_context(tc.tile_pool(name="ohp", bufs=10))
        psw = ctx.enter_context(tc.tile_pool(name="psw", bufs=2, space="PSUM"))
        psh = ctx.enter_context(tc.tile_pool(name="psh", bufs=2, space="PSUM"))
        pst = ctx.enter_context(tc.tile_pool(name="pst", bufs=2, space="PSUM"))
        sb = ctx.enter_context(tc.tile_pool(name="sb", bufs=1))

        # --- constants ---
        # per-window iota replicated over the TPS slots (dense fp16 so the
        # one-hot tensor_tensor reads step-1 data on port 0)
        iota_g = []
        for g in range(G):
            it = const.tile([128, TPS, 128], F16, name=f"iota_g{g}")
            nc.gpsimd.iota(it[:], pattern=[[0, TPS], [1, 128]], base=128 * g,
                           channel_multiplier=0,
                           allow_small_or_imprecise_dtypes=True)
            iota_g.append(it)
        pidx = const.tile([128, 1], F32)          # partition index
        nc.gpsimd.iota(pidx[:], pattern=[[0, 1]], base=0, channel_multiplier=1,
                       allow_small_or_imprecise_dtypes=True)
        identcmp = const.tile([128, 128], F32)
        nc.gpsimd.iota(identcmp[:], pattern=[[1, 128]], base=0,
                       channel_multiplier=0,
                       allow_small_or_imprecise_dtypes=True)
        ident = const.tile([128, 128], F32)       # identity for PE transpose
        nc.vector.tensor_scalar(ident[:], identcmp[:], pidx[:], None,
                                op0=mybir.AluOpType.is_equal)

        # --- weights / biases ---
        w1_sb = const.tile([128, 2, H], F32)
        nc.sync.dma_start(w1_sb[:], w1_d.ap().rearrange("(k p) h -> p k h", p=128))
        w2_sb = const.tile([128, 2, H], F32)
        nc.sync.dma_start(w2_sb[:], w2_d.ap().rearrange("(k p) h -> p k h", p=128))
        b1_sb = const.tile([128, 2], F32)
        nc.sync.dma_start(b1_sb[:], b1_d.ap().rearrange("(m p) -> p m", p=128))
        b2_sb = const.tile([128, 2], F32)
        nc.sync.dma_start(b2_sb[:], b2_d.ap().rearrange("(m p) -> p m", p=128))

        # --- all bseg values in one contiguous DMA: [128, NSUP, TPS] ---
        bseg_sb = const.tile([128, NSUP, TPS], F16)
        nc.scalar.dma_start(bseg_sb[:],
                            bs_d.ap().rearrange("p (s b) -> p s b", b=TPS))
        rcp_sb = const.tile([128, G], F32)
        nc.scalar.dma_start(rcp_sb[:], rcp_d.ap())

        # --- segment sums + counts over 4 windows ---
        pooled = sb.tile([128, G, H], F32)  # window g -> pooled[:, g, :]
        wps = {}
        for st in range(NSUP):
            # rows are laid out p-major within a supertile
            # (row = st*2048 + 16p + b) so the DMA moves 8KB-contiguous
            # runs per partition on both sides (512B packets measured
            # 18GB/s/engine; 8KB restores near-peak DMA efficiency)
            x_sb = xp.tile([128, TPS * H], F16)
            # alternate between the two HWDGE queues (Sync / Scalar)
            dma_eng = nc.sync if st % 2 == 0 else nc.scalar
            dma_eng.dma_start(
                x_sb[:],
                xs_d.ap()[st * SUP_ROWS:(st + 1) * SUP_ROWS, :]
                    .rearrange("(p b) h -> p (b h)", p=128))
            bcast = (bseg_sb[:, st, :].rearrange("p (b u) -> p b u", u=1)
                     .broadcast_to((128, TPS, 128)))
            for g in range(G):
                if not (WLO[g] <= st < WHI[g]):
                    continue
                if st == WLO[g]:
                    wps[g] = psw.tile([128, H], F32, name="wps", tag="wps")
                # one-hot for all TPS tiles of this supertile at once
                oh = ohp.tile([128, TPS, 128], F16)
                nc.vector.tensor_tensor(oh[:], iota_g[g][:], bcast,
                                        op=mybir.AluOpType.is_equal)
                for b in range(TPS):
                    nc.tensor.matmul(wps[g][:], oh[:, b, :],
                                     x_sb[:, b * H:(b + 1) * H],
                                     start=(st == WLO[g] and b == 0),
                                     stop=(st == WHI[g] - 1 and b == TPS - 1))
                if st == WHI[g] - 1:
                    # evict: pooled = sums * host-provided 1/max(cnt,1)
                    nc.vector.tensor_scalar_mul(pooled[:, g, :],
                                                wps[g][:, 0:H],
                                                rcp_sb[:, g:g + 1])

        # --- transpose pooled -> pooledT [128, 2, 512] (h-chunk, seg) ---
        pooledT = sb.tile([128, 2, SEG_PC], F32)
        for g in range(G):
            for j in range(2):
                pt = pst.tile([128, 128], F32)
                nc.tensor.transpose(pt[:], pooled[:, g, j * 128:(j + 1) * 128],
                                    ident[:])
                nc.vector.tensor_copy(pooledT[:, j, g * 128:(g + 1) * 128], pt[:])

        # --- MLP layer 1: hT = gelu(W1.T @ pooledT + b1) ---
        hT = sb.tile([128, 2, SEG_PC], F32)
        for m in range(2):
            ph = psh.tile([128, SEG_PC], F32)
            for k in range(2):
                nc.tensor.matmul(ph[:], w1_sb[:, k, m * 128:(m + 1) * 128],
                                 pooledT[:, k, :], start=(k == 0), stop=(k == 1))
            act = (mybir.ActivationFunctionType.Gelu if use_gelu
                   else mybir.ActivationFunctionType.Identity)
            nc.scalar.activation(hT[:, m, :], ph[:], act,
                                 bias=b1_sb[:, m:m + 1], scale=1.0)

        # --- MLP layer 2: oT = W2.T @ hT + b2 ---
        oT = sb.tile([128, 2, SEG_PC], F32)
        for m in range(2):
            ph = psh.tile([128, SEG_PC], F32)
            for k in range(2):
                nc.tensor.matmul(ph[:], w2_sb[:, k, m * 128:(m + 1) * 128],
                                 hT[:, k, :], start=(k == 0), stop=(k == 1))
            nc.scalar.activation(oT[:, m, :], ph[:],
                                 mybir.ActivationFunctionType.Identity,
                                 bias=b2_sb[:, m:m + 1], scale=1.0)

        # --- transpose back and store ---
        out_sb = sb.tile([128, G, H], F32)
        for g in range(G):
            for j in range(2):
                pt = pst.tile([128, 128], F32)
                nc.tensor.transpose(pt[:], oT[:, j, g * 128:(g + 1) * 128],
                                    ident[:])
                nc.vector.tensor_copy(out_sb[:, g, j * 128:(j + 1) * 128], pt[:])
        nc.sync.dma_start(out_d.ap().rearrange("(g p) h -> p g h", p=128),
                          out_sb[:])

    nc.compile()
    return nc


def _get_nc():
    global _nc_cache
    if _nc_cache is None:
        _nc_cache = _build_nc()
    return _nc_cache


def _make_in_maps(x, batch, W1, b1, W2, b2):
    # fp16 input path: PE runs fp16 matmuls at 4x the fp32 rate and DMA
    # bytes halve; accumulation stays fp32 in PSUM. Measured output rel
    # err ~1e-4 (vs ~5e-5 for the all-fp32 path).
    x16 = np.ascontiguousarray(np.asarray(x).astype(np.float16))
    batch_i = np.asarray(batch).astype(np.int64)
    W1 = np.ascontiguousarray(np.asarray(W1, dtype=np.float32))
    b1 = np.ascontiguousarray(np.asarray(b1, dtype=np.float32))
    W2 = np.ascontiguousarray(np.asarray(W2, dtype=np.float32))
    b2 = np.ascontiguousarray(np.asarray(b2, dtype=np.float32))

    bounds = np.searchsorted(batch_i, SEG_PC * np.arange(NCORES + 1))
    starts = np.minimum(bounds[:NCORES], N - R_PAD)
    starts = np.maximum(starts, 0)

    # safety: every window's rows must fall inside its processed supertiles
    wb = np.searchsorted(batch_i, np.arange(0, S + 1, 128))  # 128-seg bounds
    for k in range(NCORES):
        r = int(starts[k])
        for g in range(G):
            lo = int(wb[4 * k + g]) - r
            hi = int(wb[4 * k + g + 1]) - r
            assert lo >= WLO[g] * SUP_ROWS and hi <= WHI[g] * SUP_ROWS, (
                f"window coverage violated: core {k} window {g}: "
                f"[{lo},{hi}) not in "
                f"[{WLO[g] * SUP_ROWS},{WHI[g] * SUP_ROWS})")

    # segment counts -> 1/max(cnt,1), host side (O(N) int work, same
    # order as the bseg index preprocessing; all x compute is on device)
    cnt = np.bincount(batch_i, minlength=S).astype(np.float32)
    rcp_all = (1.0 / np.maximum(cnt, 1.0)).astype(np.float32)

    in_maps = []
    for k in range(NCORES):
        r = int(starts[k])
        bs = (batch_i[r:r + R_PAD] - SEG_PC * k).astype(np.float16)
        # on-chip layout [partition, supertile, tile-slot] with rows
        # p-major within a supertile: row = st*2048 + 16p + b
        bs = np.ascontiguousarray(
            bs.reshape(NSUP, 128, TPS).transpose(1, 0, 2).reshape(128, -1))
        rcp = np.ascontiguousarray(
            rcp_all[SEG_PC * k:SEG_PC * (k + 1)].reshape(G, 128).T)
        in_maps.append({
            "xs": x16[r:r + R_PAD],
            "bs": bs,
            "rcp": rcp,
            "w1": W1, "b1": b1, "w2": W2, "b2": b2,
        })
    return in_maps


def _run(x, batch, W1, b1, W2, b2, trace=False, **spmd_kwargs):
    in_maps = _make_in_maps(x, batch, W1, b1, W2, b2)
    nc = _get_nc()
    res = bass_utils.run_bass_kernel_spmd(
        nc, in_maps, core_ids=list(range(NCORES)), trace=trace, **spmd_kwargs)
    out = np.concatenate([res.results[k]["out"] for k in range(NCORES)], axis=0)
    return out.astype(np.float32, copy=False), res


def kernel(x, edge_index, edge_type, batch, W1, b1, W2, b2):
    out, _ = _run(x, batch, W1, b1, W2, b2)
    return out



# revision 2
# speedup vs baseline: 1.8068x; 1.8068x over previous
"""Trainium2 Bass kernel for nn_SimpleMLP (segment-mean + 2-layer MLP), v2.

reference:
  sums = segment_sum(x, batch, 4096); cnt = segment_sum(ones, batch, 4096)
  pooled = sums / max(cnt, 1);  out = gelu(pooled @ W1 + b1) @ W2 + b2

Distribution (8 cores, no collectives): `batch` is sorted, so core k owns
segments [512k, 512k+512). The host pads x rows (zero rows, <=1 per
segment) so every segment starts at an EVEN padded row index, making
every DRAM row-pair segment-pure, casts x to fp8e4, and hands core k a
fixed-size row slab plus per-PAIR segment ids.

On-device per 4096-row supertile (32 rows/partition = 8KB fp8 DMA runs):
ONE tensor_tensor is_equal per 128-segment window builds the PAIR one-hot
in fp16 on [p,16,64,2]-shaped views (all operands 2-byte packed -> DVE
2x_1p mode, ~half the baseline's vector cost since there is one compare
per pair, not per row). The fp16 one-hot (1.0 = 0x3C00) is then viewed as
fp8e4 bytes: byte 1 of each fp16 is 0x3C = 1.5 at hot positions. A
[K, i(stride 0), m(stride 2, offset 1)] fp8 view feeds DoubleRow matmuls
(fp8e4, 0.5 cyc/row): each instruction contracts a pair-column of two
128-row k-tiles sharing the broadcast one-hot. The 1.5 scale and the
count reciprocal fold into one host-side factor. Mean + per-window MLP
(fp32 matmuls, hardware Gelu) run replicated per core; host concatenates
the 8 [512, 256] outputs.
"""
import sys

sys.path.insert(0, "/opt/trn_rl_repo")

from contextlib import ExitStack

import ml_dtypes
import numpy as np

import concourse.bacc as bacc
import concourse.mybir as mybir
import concourse.tile as tile
from concourse import bass_utils

F32 = mybir.dt.float32
F16 = mybir.dt.float16
F8 = mybir.dt.float8e4

N = 1048576
H = 256
S = 4096
NCORES = 8
SEG_PC = S // NCORES          # 512 segments per core
G = 4                         # 128-seg windows per core
TPS = 32                      # row-slots per partition per supertile
SUP_ROWS = TPS * 128          # 4096
NSUP = 33
R_PAD = NSUP * SUP_ROWS       # 135168 rows per core slab
NPAIR_ST = TPS // 2           # 16 DoubleRow groups per supertile

# window g covers local segs [128g, 128g+128); processed supertile range
# [8g-1, 8g+10) covers data-dependent row drift plus core-7's clamp shift
# (up to ~4100 rows) and the even-alignment padding drift.
WLO = [max(8 * g - 1, 0) for g in range(G)]
WHI = [min(8 * g + 10, NSUP) for g in range(G)]

_nc_cache = None


def _build_nc():
    nc = bacc.Bacc("TRN2", target_bir_lowering=False, debug=False,
                   num_devices=NCORES)
    xs_d = nc.dram_tensor("xs", [R_PAD, H], F8, kind="ExternalInput")
    # per-PAIR local segment id, duplicated x2, on-chip layout
    # [p, st, d, 2] (pair (p,d) of supertile st = rows st*4096+32p+2d+{0,1})
    bs_d = nc.dram_tensor("bs", [128, NSUP, NPAIR_ST, 2], F16,
                          kind="ExternalInput")
    # 1/(1.5*max(cnt,1)) per segment: [p, g] for window g
    rcp_d = nc.dram_tensor("rcp", [128, G], F32, kind="ExternalInput")
    w1_d = nc.dram_tensor("w1", [H, H], F32, kind="ExternalInput")
    b1_d = nc.dram_tensor("b1", [H], F32, kind="ExternalInput")
    w2_d = nc.dram_tensor("w2", [H, H], F32, kind="ExternalInput")
    b2_d = nc.dram_tensor("b2", [H], F32, kind="ExternalInput")
    out_d = nc.dram_tensor("out", [SEG_PC, H], F32, kind="ExternalOutput")

    with tile.TileContext(nc) as tc, ExitStack() as ctx:
        const = ctx.enter_context(tc.tile_pool(name="const", bufs=1))
        xp = ctx.enter_context(tc.tile_pool(name="xp", bufs=8))
        ohp = ctx.enter_context(tc.tile_pool(name="ohp", bufs=8))
        psw = ctx.enter_context(tc.tile_pool(name="psw", bufs=2, space="PSUM"))
        psh = ctx.enter_context(tc.tile_pool(name="psh", bufs=2, space="PSUM"))
        pst = ctx.enter_context(tc.tile_pool(name="pst", bufs=2, space="PSUM"))
        sb = ctx.enter_context(tc.tile_pool(name="sb", bufs=1))

        # --- constants ---
        iota_g = []
        for g in range(G):
            it = const.tile([128, NPAIR_ST, 128], F16, name=f"iota_g{g}")
            nc.gpsimd.iota(it[:], pattern=[[0, NPAIR_ST], [1, 128]],
                           base=128 * g, channel_multiplier=0,
                           allow_small_or_imprecise_dtypes=True)
            iota_g.append(it)
        pidx = const.tile([128, 1], F32)          # partition index
        nc.gpsimd.iota(pidx[:], pattern=[[0, 1]], base=0, channel_multiplier=1,
                       allow_small_or_imprecise_dtypes=True)
        identcmp = const.tile([128, 128], F32)
        nc.gpsimd.iota(identcmp[:], pattern=[[1, 128]], base=0,
                       channel_multiplier=0,
                       allow_small_or_imprecise_dtypes=True)
        ident = const.tile([128, 128], F32)       # identity for PE transpose
        nc.vector.tensor_scalar(ident[:], identcmp[:], pidx[:], None,
                                op0=mybir.AluOpType.is_equal)

        # --- weights / biases ---
        w1_sb = const.tile([128, 2, H], F32)
        nc.sync.dma_start(w1_sb[:], w1_d.ap().rearrange("(k p) h -> p k h", p=128))
        w2_sb = const.tile([128, 2, H], F32)
        nc.sync.dma_start(w2_sb[:], w2_d.ap().rearrange("(k p) h -> p k h", p=128))
        b1_sb = const.tile([128, 2], F32)
        nc.sync.dma_start(b1_sb[:], b1_d.ap().rearrange("(m p) -> p m", p=128))
        b2_sb = const.tile([128, 2], F32)
        nc.sync.dma_start(b2_sb[:], b2_d.ap().rearrange("(m p) -> p m", p=128))

        bs_sb = const.tile([128, NSUP, NPAIR_ST, 2], F16)
        nc.scalar.dma_start(bs_sb[:], bs_d.ap())
        rcp_sb = const.tile([128, G], F32)
        nc.scalar.dma_start(rcp_sb[:], rcp_d.ap())

        out_sb = sb.tile([128, G, H], F32)

        def window_mlp(g, pooled_g):
            # pooled_g: [128 segs, 256] f32 for window g -> out_sb[:, g, :]
            pooledT = sb.tile([128, 2, 128], F32, name="pooledT", tag="pT")
            for j in range(2):
                pt = pst.tile([128, 128], F32)
                nc.tensor.transpose(pt[:], pooled_g[:, j * 128:(j + 1) * 128],
                                    ident[:])
                nc.vector.tensor_copy(pooledT[:, j, :], pt[:])
            hT = sb.tile([128, 2, 128], F32, name="hT", tag="hT")
            for m in range(2):
                ph = psh.tile([128, 128], F32)
                for k in range(2):
                    nc.tensor.matmul(ph[:], w1_sb[:, k, m * 128:(m + 1) * 128],
                                     pooledT[:, k, :], start=(k == 0),
                                     stop=(k == 1))
                nc.scalar.activation(hT[:, m, :], ph[:],
                                     mybir.ActivationFunctionType.Gelu,
                                     bias=b1_sb[:, m:m + 1], scale=1.0)
            oT = sb.tile([128, 2, 128], F32, name="oT", tag="oT")
            for m in range(2):
                ph = psh.tile([128, 128], F32)
                for k in range(2):
                    nc.tensor.matmul(ph[:], w2_sb[:, k, m * 128:(m + 1) * 128],
                                     hT[:, k, :], start=(k == 0), stop=(k == 1))
                nc.scalar.activation(oT[:, m, :], ph[:],
                                     mybir.ActivationFunctionType.Identity,
                                     bias=b2_sb[:, m:m + 1], scale=1.0)
            for j in range(2):
                pt = pst.tile([128, 128], F32)
                nc.tensor.transpose(pt[:], oT[:, j, :], ident[:])
                nc.vector.tensor_copy(out_sb[:, g, j * 128:(j + 1) * 128],
                                      pt[:])

        # --- segment sums over 4 windows ---
        wps = {}
        for st in range(NSUP):
            # rows p-major within a supertile (row = st*4096 + 32p + 2d + i)
            # -> one 8KB-contiguous run per partition per supertile
            x_sb = xp.tile([128, NPAIR_ST, 2, H], F8)
            dma_eng = nc.sync if st % 2 == 0 else nc.scalar
            dma_eng.dma_start(
                x_sb[:],
                xs_d.ap()[st * SUP_ROWS:(st + 1) * SUP_ROWS, :]
                    .rearrange("(p d i) h -> p d i h", p=128, d=NPAIR_ST))
            bs_v = (bs_sb[:, st, :, :]
                    .rearrange("p d (u l) -> p d u l", u=1)
                    .broadcast_to((128, NPAIR_ST, 64, 2)))
            for g in range(G):
                if not (WLO[g] <= st < WHI[g]):
                    continue
                if st == WLO[g]:
                    wps[g] = psw.tile([128, H], F32, name="wps", tag="wps")
                # pair one-hot for the whole supertile, fp16, 2x_1p views
                oh16 = ohp.tile([128, NPAIR_ST, 128], F16)
                oh_v = oh16[:].rearrange("p d (j l) -> p d j l", l=2)
                iota_v = iota_g[g][:].rearrange("p d (j l) -> p d j l", l=2)
                nc.vector.tensor_tensor(oh_v, iota_v, bs_v,
                                        op=mybir.AluOpType.is_equal)
                oh8 = oh16[:].bitcast(F8)  # [128, NPAIR_ST, 256]
                for d in range(NPAIR_ST):
                    lhsT = (oh8[:, d, :]
                            .rearrange("p (m l) -> p m l", l=2)[:, :, 1:2]
                            .rearrange("p m (u) -> p u m", u=1)
                            .broadcast_to((128, 2, 128)))
                    nc.tensor.matmul(
                        wps[g][:], lhsT, x_sb[:, d, :, :],
                        start=(st == WLO[g] and d == 0),
                        stop=(st == WHI[g] - 1 and d == NPAIR_ST - 1),
                        perf_mode=mybir.MatmulPerfMode.DoubleRow)
                if st == WHI[g] - 1:
                    pooled_g = sb.tile([128, H], F32, name="pooled", tag="pl")
                    nc.vector.tensor_scalar_mul(pooled_g[:], wps[g][:],
                                                rcp_sb[:, g:g + 1])
                    window_mlp(g, pooled_g)

        nc.sync.dma_start(out_d.ap().rearrange("(g p) h -> p g h", p=128),
                          out_sb[:])

    nc.compile()
    return nc


def _get_nc():
    global _nc_cache
    if _nc_cache is None:
        _nc_cache = _build_nc()
    return _nc_cache


def _even_pad_layout(batch_i):
    """Padded row layout: every segment starts at an even padded index.

    Returns (newpos[N] padded position of each original row,
             pstart[S+1] padded start of each segment, NP total padded rows).
    """
    cnt = np.bincount(batch_i, minlength=S).astype(np.int64)
    step = cnt + (cnt & 1)                     # per-segment padded length
    pstart = np.zeros(S + 1, np.int64)
    np.cumsum(step, out=pstart[1:])
    orig_start = np.zeros(S + 1, np.int64)
    np.cumsum(cnt, out=orig_start[1:])
    shift = pstart[:S] - orig_start[:S]        # per-segment shift
    newpos = np.arange(N, dtype=np.int64) + shift[batch_i]
    return newpos, pstart, int(pstart[S]), cnt


def _make_in_maps(x, batch, W1, b1, W2, b2):
    batch_i = np.asarray(batch).astype(np.int64)
    W1 = np.ascontiguousarray(np.asarray(W1, dtype=np.float32))
    b1 = np.ascontiguousarray(np.asarray(b1, dtype=np.float32))
    W2 = np.ascontiguousarray(np.asarray(W2, dtype=np.float32))
    b2 = np.ascontiguousarray(np.asarray(b2, dtype=np.float32))

    newpos, pstart, NP, cnt = _even_pad_layout(batch_i)

    # fp8 padded x (pad rows zero; they pair with their segment's tail row).
    # x16 scaling pushes small values out of the fp8 denormal range (the PE
    # flushes fp8 denormals); max |x|*16 ~ 87 < 240 so no saturation.
    xp8 = np.zeros((NP, H), ml_dtypes.float8_e4m3)
    xp8[newpos] = (np.asarray(x) * np.float32(16.0)).astype(
        ml_dtypes.float8_e4m3)
    # padded local-ish segment ids (pad rows never read: pairs read even idx)
    bp = np.zeros(NP, np.int64)
    bp[newpos] = batch_i

    starts = np.minimum(pstart[SEG_PC * np.arange(NCORES)], NP - R_PAD)
    starts = np.maximum(starts, 0) & ~np.int64(1)   # even-align slab starts

    # safety: every window's padded rows must fall inside its supertiles
    for k in range(NCORES):
        r = int(starts[k])
        for g in range(G):
            lo = int(pstart[SEG_PC * k + 128 * g]) - r
            hi = int(pstart[SEG_PC * k + 128 * (g + 1)]) - r
            lo = max(lo, 0)
            hi = min(hi, R_PAD)
            assert lo >= WLO[g] * SUP_ROWS and hi <= WHI[g] * SUP_ROWS, (
                f"window coverage violated: core {k} window {g}: "
                f"[{lo},{hi}) not in "
                f"[{WLO[g] * SUP_ROWS},{WHI[g] * SUP_ROWS})")

    # 1/(24 * max(cnt,1)): 1.5 = fp8e4 value of the fp16(1.0) high byte,
    # 16 = host-side x prescale
    rcp_all = (1.0 / (24.0 * np.maximum(cnt, 1.0))).astype(np.float32)

    in_maps = []
    for k in range(NCORES):
        r = int(starts[k])
        pair_seg = (bp[r:r + R_PAD:2] - SEG_PC * k).astype(np.float16)
        # [NSUP*2048 pairs] -> [p, st, d, 2]: pair j = st*2048 + 16p + d
        bs = pair_seg.reshape(NSUP, 128, NPAIR_ST).transpose(1, 0, 2)
        bs = np.ascontiguousarray(
            np.repeat(bs[:, :, :, None], 2, axis=3))
        rcp = np.ascontiguousarray(
            rcp_all[SEG_PC * k:SEG_PC * (k + 1)].reshape(G, 128).T)
        in_maps.append({
            "xs": xp8[r:r + R_PAD],
            "bs": bs,
            "rcp": rcp,
            "w1": W1, "b1": b1, "w2": W2, "b2": b2,
        })
    return in_maps


def _run(x, batch, W1, b1, W2, b2, trace=False, **spmd_kwargs):
    in_maps = _make_in_maps(x, batch, W1, b1, W2, b2)
    nc = _get_nc()
    res = bass_utils.run_bass_kernel_spmd(
        nc, in_maps, core_ids=list(range(NCORES)), trace=trace, **spmd_kwargs)
    out = np.concatenate([res.results[k]["out"] for k in range(NCORES)], axis=0)
    return out.astype(np.float32, copy=False), res


def kernel(x, edge_index, edge_type, batch, W1, b1, W2, b2):
    out, _ = _run(x, batch, W1, b1, W2, b2)
    return out
